# revision 10
# baseline (speedup 1.0000x reference)
"""Trainium2 Bass kernel for nn_De_conv_batched_multimasks (segment_reduce).

Self-contained: accepts FULL inputs, shards areas across 8 NeuronCores
(fully data-parallel over the B*N supervoxel areas), runs one SPMD Bass
kernel, gathers the full [B, N] output.

Math notes (exact reformulations of the reference):
 - or_simple(x, y) = 2x - x^2 (y-independent)  =>  eroded = (1 - edge_diff*mc)^2 * mc
   with edge_diff = a_b + a_f - 2*a_b*a_f (only row shifts matter).
 - Every sin argument in the diff_round chains lies in [0,1] or [-1,0], so
   sin(2*pi*x) is evaluated as sin(2*pi*x -+ pi) = -sin(2*pi*x) with the
   +-pi put into the ACT bias (free), keeping the spline in its valid range.
 - The "+1" offsets (t = ... + 1) ride on sin periodicity: chains track t-1.
"""

import numpy as np

import concourse.bacc as bacc
import concourse.mybir as mybir
from concourse.tile import TileContext
from concourse.bass_utils import run_bass_kernel_spmd

F32 = mybir.dt.float32
AX = mybir.AxisListType
OP = mybir.AluOpType
ACTF = mybir.ActivationFunctionType

PI = float(np.pi)
TWO_PI = 2.0 * PI
INV_2PI = 1.0 / TWO_PI
EPS = 1e-8

B, N, W, H = 8, 8192, 8, 8
PX = W * H                  # 64
CH = 4
NCORES = 8
A_TOT = B * N               # 65536
A_CORE = A_TOT // NCORES    # 8192
P = 128
G = 16                      # areas per partition per chunk
A_CHUNK = P * G             # 2048
CHUNKS = A_CORE // A_CHUNK  # 4


def _build():
    nc = bacc.Bacc("TRN2", target_bir_lowering=False, debug=False,
                   num_devices=NCORES)
    img_d = nc.dram_tensor("img", [A_CORE, PX], F32, kind="ExternalInput")
    mask_d = nc.dram_tensor("mask", [A_CORE, PX * CH], F32, kind="ExternalInput")
    mid_d = nc.dram_tensor("mid", [A_CORE, CH], F32, kind="ExternalInput")
    edge_d = nc.dram_tensor("edge", [A_CORE, PX], F32, kind="ExternalInput")
    out_d = nc.dram_tensor("out", [A_CORE], F32, kind="ExternalOutput")

    # DRAM views: area a = c*A_CHUNK + p*G + g  (contiguous per partition)
    img_v = img_d.ap().rearrange("(c p g) x -> c p (g x)", p=P, g=G)
    mask_v = mask_d.ap().rearrange("(c p g) x -> c p (g x)", p=P, g=G)
    mid_v = mid_d.ap().rearrange("(c p g) x -> c p (g x)", p=P, g=G)
    edge_v = edge_d.ap().rearrange("(c p g) x -> c p (g x)", p=P, g=G)
    out_v = out_d.ap().rearrange("(c p g) -> c p g", p=P, g=G)

    FD = G * PX * CH        # 4096 mask-path free dim
    FE = G * PX             # 1024 pixel-path free dim

    with TileContext(nc) as tc:
        with tc.tile_pool(name="cpool", bufs=1) as cpool, \
             tc.tile_pool(name="pool", bufs=2) as pool:
            bias_n = cpool.tile([P, 1], F32)   # -pi
            bias_p = cpool.tile([P, 1], F32)   # +pi
            nc.vector.memset(bias_n[:, :], -PI)
            nc.vector.memset(bias_p[:, :], PI)
            BN = bias_n[:, :]
            BP = bias_p[:, :]
            for c in range(CHUNKS):
                # ---- loads
                mask_t = pool.tile([P, FD], F32)
                img_t = pool.tile([P, FE], F32)
                edge_t = pool.tile([P, FE], F32)
                mid_t = pool.tile([P, G * CH], F32)
                nc.sync.dma_start(mask_t[:, :], mask_v[c])
                nc.sync.dma_start(img_t[:, :], img_v[c])
                nc.sync.dma_start(edge_t[:, :], edge_v[c])
                nc.sync.dma_start(mid_t[:, :], mid_v[c])

                # ---- compact mid chain: b2 = hdr(mid), b2p = 2*b2 - 1
                sm = pool.tile([P, G * CH], F32)
                m1 = pool.tile([P, G * CH], F32)
                b2 = pool.tile([P, G * CH], F32)
                b2p = pool.tile([P, G * CH], F32)
                nc.scalar.activation(sm[:, :], mid_t[:, :], ACTF.Sin,
                                     scale=TWO_PI, bias=BN)
                nc.vector.scalar_tensor_tensor(m1[:, :], sm[:, :], INV_2PI,
                                               mid_t[:, :], op0=OP.mult, op1=OP.add)
                nc.scalar.activation(sm[:, :], m1[:, :], ACTF.Sin,
                                     scale=TWO_PI, bias=BN)
                nc.vector.scalar_tensor_tensor(b2[:, :], sm[:, :], INV_2PI,
                                               m1[:, :], op0=OP.mult, op1=OP.add)
                nc.vector.tensor_scalar(b2p[:, :], b2[:, :], 2.0, -1.0,
                                        op0=OP.mult, op1=OP.add)
                # ---- mask-path chain (3 big tiles: mask_t, sA, cA)
                sA = pool.tile([P, FD], F32)
                cA = pool.tile([P, FD], F32)

                def v4(t):
                    return t[:, :].rearrange("p (g x c) -> p g x c", g=G, c=CH)

                # s1 = -sin(2pi*mask); a1 = dr(mask) = mask + s1/2pi
                nc.scalar.activation(sA[:, :], mask_t[:, :], ACTF.Sin,
                                     scale=TWO_PI, bias=BN)
                nc.vector.scalar_tensor_tensor(cA[:, :], sA[:, :], INV_2PI,
                                               mask_t[:, :], op0=OP.mult, op1=OP.add)
                # s2 -> mask_t; a2 -> sA
                nc.scalar.activation(mask_t[:, :], cA[:, :], ACTF.Sin,
                                     scale=TWO_PI, bias=BN)
                nc.vector.scalar_tensor_tensor(sA[:, :], mask_t[:, :], INV_2PI,
                                               cA[:, :], op0=OP.mult, op1=OP.add)
                # ta = a2*b2p -> cA ; tb = t-1 = ta - b2 -> mask_t
                # (per-channel: STT/TT APs are limited to 3D)
                b2_g = b2[:, :].rearrange("p (g c) -> p g c", g=G)
                b2p_g = b2p[:, :].rearrange("p (g c) -> p g c", g=G)
                for ci in range(CH):
                    bc1 = b2p_g[:, :, ci].unsqueeze(2).broadcast_to([P, G, PX])
                    bc0 = b2_g[:, :, ci].unsqueeze(2).broadcast_to([P, G, PX])
                    nc.vector.tensor_tensor(v4(cA)[:, :, :, ci],
                                            v4(sA)[:, :, :, ci], bc1, op=OP.mult)
                    nc.vector.scalar_tensor_tensor(v4(mask_t)[:, :, :, ci],
                                                   bc0, -1.0,
                                                   v4(cA)[:, :, :, ci],
                                                   op0=OP.mult, op1=OP.add)
                # s3 -> sA ; t1 -> cA     (tb in [-1,0] => bias +pi)
                nc.scalar.activation(sA[:, :], mask_t[:, :], ACTF.Sin,
                                     scale=TWO_PI, bias=BP)
                nc.vector.scalar_tensor_tensor(cA[:, :], sA[:, :], INV_2PI,
                                               mask_t[:, :], op0=OP.mult, op1=OP.add)
                # s4 -> sA ; cb -> mask_t
                nc.scalar.activation(sA[:, :], cA[:, :], ACTF.Sin,
                                     scale=TWO_PI, bias=BP)
                nc.vector.scalar_tensor_tensor(mask_t[:, :], sA[:, :], INV_2PI,
                                               cA[:, :], op0=OP.mult, op1=OP.add)
                # s5 -> sA ; db -> cA
                nc.scalar.activation(sA[:, :], mask_t[:, :], ACTF.Sin,
                                     scale=TWO_PI, bias=BP)
                nc.vector.scalar_tensor_tensor(cA[:, :], sA[:, :], INV_2PI,
                                               mask_t[:, :], op0=OP.mult, op1=OP.add)
                # d = db + 1 -> mask_t
                nc.vector.tensor_scalar(mask_t[:, :], cA[:, :], 1.0, None,
                                        op0=OP.add)

                # ---- and-tree
                q = pool.tile([P, G * PX * 2], F32)
                r = pool.tile([P, G * PX * 2], F32)
                d_v = v4(mask_t)
                q_v = q[:, :].rearrange("p (g x u) -> p g x u", g=G, u=2)
                nc.vector.tensor_tensor(q_v[:, :, :, 0], d_v[:, :, :, 0],
                                        d_v[:, :, :, 1], op=OP.mult)
                nc.vector.tensor_tensor(q_v[:, :, :, 1], d_v[:, :, :, 2],
                                        d_v[:, :, :, 3], op=OP.mult)
                sq = pool.tile([P, G * PX * 2], F32)
                nc.scalar.activation(sq[:, :], q[:, :], ACTF.Sin,
                                     scale=TWO_PI, bias=BN)
                nc.vector.scalar_tensor_tensor(r[:, :], sq[:, :], INV_2PI,
                                               q[:, :], op0=OP.mult, op1=OP.add)
                r_v = r[:, :].rearrange("p (g x u) -> p g x u", g=G, u=2)

                # mc into padded tile [P, G, 10, 8] (rows 0 and 9 zero)
                mcp = pool.tile([P, G * 80], F32)
                mcp_v = mcp[:, :].rearrange("p (g w h) -> p g w h", g=G, w=10)
                mcp3 = mcp[:, :].rearrange("p (g e) -> p g e", g=G)
                nc.vector.memset(mcp_v[:, :, 0, :], 0.0)
                nc.vector.memset(mcp_v[:, :, 9, :], 0.0)
                mcc = mcp3[:, :, 8:72]             # [P, G, 64] center rows 1..8
                nc.vector.tensor_tensor(mcc, r_v[:, :, :, 0], r_v[:, :, :, 1],
                                        op=OP.mult)

                # ---- erosion
                ab = mcp3[:, :, 16:80]
                af = mcp3[:, :, 0:64]
                e1 = pool.tile([P, FE], F32)
                e2 = pool.tile([P, FE], F32)
                hh = pool.tile([P, FE], F32)

                def vE(t):
                    return t[:, :].rearrange("p (g x) -> p g x", g=G)

                nc.vector.tensor_tensor(vE(e1), ab, af, op=OP.mult)
                nc.vector.tensor_tensor(vE(e2), ab, af, op=OP.add)
                nc.vector.scalar_tensor_tensor(hh[:, :], e1[:, :], -2.0,
                                               e2[:, :], op0=OP.mult, op1=OP.add)
                nc.vector.tensor_tensor(vE(e2), vE(hh), mcc, op=OP.mult)   # x
                nc.scalar.activation(e1[:, :], e2[:, :], ACTF.Square,
                                     scale=-1.0, bias=1.0)                 # (1-x)^2
                nc.vector.tensor_tensor(vE(hh), mcc, vE(edge_t), op=OP.mult)  # me
                nc.vector.tensor_tensor(e2[:, :], e1[:, :], hh[:, :], op=OP.mult)  # w2
                sw2 = pool.tile([P, G], F32)
                nc.vector.reduce_sum(sw2[:, :], vE(e2), axis=AX.X)

                # ---- stats
                z = pool.tile([P, FE], F32)
                nc.vector.tensor_tensor(vE(z), mcc, vE(img_t), op=OP.mult)
                sz = pool.tile([P, G], F32)
                smc = pool.tile([P, G], F32)
                nc.vector.reduce_sum(sz[:, :], vE(z), axis=AX.X)
                nc.vector.reduce_sum(smc[:, :], mcc, axis=AX.X)
                rec = pool.tile([P, G], F32)
                nc.vector.tensor_scalar(rec[:, :], smc[:, :], EPS, None, op0=OP.add)
                nc.vector.reciprocal(rec[:, :], rec[:, :])
                meann = pool.tile([P, G], F32)
                nc.vector.tensor_tensor(meann[:, :], sz[:, :], rec[:, :], op=OP.mult)
                meann_bc = meann[:, :].unsqueeze(2).broadcast_to([P, G, PX])
                nc.vector.scalar_tensor_tensor(vE(hh), meann_bc, -1.0, vE(z),
                                               op0=OP.mult, op1=OP.add)  # y0 -> hh
                nc.vector.tensor_tensor(vE(e1), vE(hh), mcc, op=OP.mult)  # y1 -> e1
                nc.scalar.activation(e2[:, :], e1[:, :], ACTF.Square)     # y2
                sy2 = pool.tile([P, G], F32)
                nc.vector.reduce_sum(sy2[:, :], vE(e2), axis=AX.X)
                varr = pool.tile([P, G], F32)
                nc.vector.tensor_tensor(varr[:, :], sy2[:, :], rec[:, :], op=OP.mult)
                outv = pool.tile([P, G], F32)
                nc.vector.scalar_tensor_tensor(outv[:, :], varr[:, :],
                                               1000.0 / PX, sw2[:, :],
                                               op0=OP.mult, op1=OP.mult)
                nc.sync.dma_start(out_v[c], outv[:, :])
    nc.compile()
    return nc


_NC = None


def _get_nc():
    global _NC
    if _NC is None:
        _NC = _build()
    return _NC


def kernel(resized_image, mask_combined, initial_masks, edge_map, mask_index=0):
    img = np.ascontiguousarray(resized_image, dtype=np.float32).reshape(A_TOT, PX)
    mask = np.ascontiguousarray(mask_combined, dtype=np.float32).reshape(A_TOT, PX * CH)
    mid = np.ascontiguousarray(initial_masks, dtype=np.float32).reshape(A_TOT, CH)
    edge = np.ascontiguousarray(edge_map, dtype=np.float32).reshape(A_TOT, PX)

    nc = _get_nc()
    in_maps = []
    for k in range(NCORES):
        sl = slice(k * A_CORE, (k + 1) * A_CORE)
        in_maps.append({
            "img": np.ascontiguousarray(img[sl]),
            "mask": np.ascontiguousarray(mask[sl]),
            "mid": np.ascontiguousarray(mid[sl]),
            "edge": np.ascontiguousarray(edge[sl]),
        })
    res = run_bass_kernel_spmd(nc, in_maps, core_ids=list(range(NCORES)))
    out = np.concatenate([res.results[k]["out"].reshape(-1)
                          for k in range(NCORES)])
    return out.reshape(B, N).astype(np.float32)
